# revision 17
# baseline (speedup 1.0000x reference)
"""HMM window log-likelihood on 8 NeuronCores (data-parallel over batch).

Math (per batch column b): y[b] = exp(logsumexp_i x_T[b,i]) with a log-space
forward recursion; evaluated here as a linear-space BACKWARD recursion
    beta_L = 1;  c_t = em_t . beta_t;  beta_{t-1} = Wn_t^T c_t
    y[b] = sum_i c_0[i,b]
where Wn_t = row-softmax(w[t-1]) (rowsum folded on host into wt) and
em_t[i,b] = L[i, bin(b,t)] * g[t], with per-step rescale scalars g[t]
(host-computed from batch column 0 in f64) folded into the 0/1 indicator
tensor G: em_t = dlt^T G_t is computed ON the PE (one K=10 matmul per
step), evacuated PSUM->SBUF bf16 by the otherwise-idle Scalar engine, and
consumed by ONE DVE multiply per 256-column chain (c = em_sb * beta_ps;
the DVE may read only one PSUM operand).  Two independent chains pipeline
the serial recursion across PE/Scalar/DVE; small dummy matmuls pin the
Tensor engine's DVFS p-state so the beta matmuls on the critical path
stay short.  Device returns colsum[b]; host: lnY = log(colsum)+C.
True lnY ~ -584.6 underflows f32 to 0.0, exactly matching the reference.
"""
import sys, os
for p in ("/opt/trn_rl_repo",):
    if p not in sys.path:
        sys.path.insert(0, p)
import numpy as np
import ml_dtypes

from concourse import bass, bacc, mybir
from concourse.tile import TileContext
from concourse.bass_utils import run_bass_kernel_spmd

W, L, B, NB = 128, 256, 4096, 10
NCORES = 8
BC = B // NCORES          # 512 batch cols per core
CHAINS = (
    ("A", 0, 256),
    ("B", 256, 512),
)
TEB = 16                  # G-block steps per DMA tile
DUMW = int(os.environ.get("KERNEL_DUMW", "160"))   # dummy matmul width
DUMN = int(os.environ.get("KERNEL_DUMN", "4"))     # dummy matmuls per step

LAST_LNY = None           # debug: device-derived lnY per batch col
LAST_RESULTS = None       # debug: raw BassKernelResults

_CACHED = None            # (nc,) build cache


def _build_nc():
    nc = bacc.Bacc("TRN2", target_bir_lowering=False, debug=False,
                   num_devices=NCORES)
    bf16, f32 = mybir.dt.bfloat16, mybir.dt.float32
    Copy = mybir.ActivationFunctionType.Copy

    wt = nc.dram_tensor("wt", [W, L - 1, W], bf16, kind="ExternalInput")
    dlt = nc.dram_tensor("dlt", [NB, W], bf16, kind="ExternalInput")
    g10 = nc.dram_tensor("g10", [NB, L, BC], bf16, kind="ExternalInput")
    ones = nc.dram_tensor("ones", [W, 1], bf16, kind="ExternalInput")
    colsum = nc.dram_tensor("colsum", [1, BC], f32, kind="ExternalOutput")

    with TileContext(nc) as tc:
        with tc.sbuf_pool(name="sb", bufs=2) as sb, \
                tc.psum_pool(name="ps", bufs=2) as ps:
            ones_sb = sb.tile([W, 1], bf16, bufs=1)
            nc.sync.dma_start(ones_sb, ones.ap())
            dlt_sb = sb.tile([NB, W], bf16, bufs=1)
            nc.sync.dma_start(dlt_sb, dlt.ap())
            dum_sb = sb.tile([W, DUMW], bf16, bufs=1)
            nc.gpsimd.memset(dum_sb, 0.0)

            # wt resident; chunked DMAs backward so the scan's first steps
            # land first.  The first G blocks are interleaved between the
            # early wt chunks so the recursion isn't stuck behind the full
            # 8 MB wt transfer.
            wt_sb = sb.tile([W, L - 1, W], bf16, bufs=1)
            g_tiles = {}

            def dma_g_block(blk):
                g_sb = sb.tile([NB, TEB, BC], bf16, tag="g", bufs=3)
                nc.sync.dma_start(
                    g_sb, g10.ap()[:, blk * TEB:(blk + 1) * TEB, :])
                for ti in range(TEB):
                    g_tiles[blk * TEB + ti] = (g_sb, ti)

            NBLK = L // TEB
            dma_g_block(NBLK - 1)
            for cc in range((L - 1 + 7) // 8 - 1, -1, -1):
                t0 = cc * 8
                cnt = min(8, L - 1 - t0)
                nc.sync.dma_start(wt_sb[:, t0:t0 + cnt, :],
                                  wt.ap()[:, t0:t0 + cnt, :])
                if cc == (L - 1 + 7) // 8 - 3:
                    dma_g_block(NBLK - 2)

            dum_ps = ps.tile([W, DUMW], f32, tag="dum", bufs=1)

            def dummy_mm():
                # p-state filler: result never read; WAW on dum_ps only
                nc.tensor.matmul(dum_ps, wt_sb[:, L - 2, :],
                                 dum_sb, start=True, stop=True,
                                 skip_group_check=True)

            def em_make(t):
                """em matmul (PE) + PSUM->SBUF bf16 evac (Scalar)."""
                g_sb, ti = g_tiles.pop(t)
                e_ps = ps.tile([W, BC], f32, tag="em_ps", bufs=2)
                nc.tensor.matmul(e_ps, dlt_sb, g_sb[:, ti, :],
                                 start=True, stop=True)
                e_sb = sb.tile([W, BC], bf16, tag="em_sb", bufs=3)
                nc.scalar.activation(e_sb, e_ps, Copy)
                return e_sb

            # pre-ramp the PE while input DMAs land
            for _ in range(8):
                dummy_mm()

            beta_ps = {}
            em_next = None
            cs_ps = ps.tile([1, BC], f32, tag="cs", bufs=1)
            for blk in range(NBLK - 1, -1, -1):
                if blk > 0 and (blk - 1) * TEB not in g_tiles:
                    dma_g_block(blk - 1)
                for ti in range(TEB - 1, -1, -1):
                    t = blk * TEB + ti
                    if t == L - 1:
                        em_next = em_make(t)
                    em_sb, em_next = em_next, None
                    c_sb = {}
                    for name, lo, hi in CHAINS:
                        c = sb.tile([W, hi - lo], bf16, tag=f"c{name}",
                                    bufs=2)
                        if t == L - 1:
                            nc.vector.tensor_copy(c, em_sb[:, lo:hi])
                        else:
                            nc.vector.tensor_mul(c, em_sb[:, lo:hi],
                                                 beta_ps[name])
                        c_sb[name] = c
                    # emission for the NEXT (t-1) step: useful PE filler
                    # while the beta matmuls below wait on the multiplies
                    if t > 0:
                        em_next = em_make(t - 1)
                    for name, lo, hi in CHAINS:
                        if t > 0:
                            b_ps = ps.tile([W, hi - lo], f32, tag=f"b{name}",
                                           bufs=2)
                            nc.tensor.matmul(b_ps, wt_sb[:, t - 1, :],
                                             c_sb[name], start=True,
                                             stop=True)
                            beta_ps[name] = b_ps
                        else:
                            nc.tensor.matmul(cs_ps[:, lo:hi], ones_sb,
                                             c_sb[name], start=True,
                                             stop=True)
                    for _ in range(DUMN if DUMW else 0):
                        dummy_mm()

            cs_sb = sb.tile([1, BC], f32, bufs=1)
            nc.vector.tensor_copy(cs_sb, cs_ps)
            nc.sync.dma_start(colsum.ap(), cs_sb)
    nc.compile()
    return nc


def _host_prep(data, input_distros, dense_layer_weights):
    f64 = np.float64
    w = dense_layer_weights.astype(f64)                    # (255,W,W)
    w = w - w.max(axis=2, keepdims=True)
    we = np.exp(w)
    wn = we / we.sum(axis=2, keepdims=True)                # row-softmax
    d = input_distros.astype(f64)
    d = d - d.max(axis=1, keepdims=True)
    e = np.exp(d)
    Ls = e / e.sum(axis=1, keepdims=True)                  # (W,NB) softmax rows
    # bins exactly as reference: floor(v / 0.1) in f32
    bins = np.minimum(NB - 1, np.floor(
        data / np.float32(0.1)).astype(np.int32))          # (B,L)

    # column-0 f64 backward pass -> per-step rescale g[t], offset C
    beta = np.ones(W, dtype=f64)
    Cacc = 0.0
    g = np.ones(L, dtype=f64)
    for t in range(L - 1, 0, -1):
        c = Ls[np.arange(W), bins[0, t]] * beta
        tmp = wn[t - 1].T @ c
        f = tmp.max()
        g[t] = 1.0 / f
        Cacc += np.log(f)
        beta = tmp * g[t]

    wt = np.ascontiguousarray(
        wn.transpose(1, 0, 2)).astype(ml_dtypes.bfloat16)  # (W,255,W)

    dL = Ls.copy()
    dL[:, 1:] -= Ls[:, :-1]
    dlt = np.ascontiguousarray(dL.T).astype(ml_dtypes.bfloat16)  # (NB,W)

    # G[k,t,b] = g[t] * [bins[b,t] >= k]   (per-step rescale folded in)
    g10 = ((bins.T[None, :, :] >= np.arange(NB)[:, None, None])
           * g[None, :, None]).astype(ml_dtypes.bfloat16)  # (NB,L,B)
    ones_v = np.ones((W, 1), dtype=ml_dtypes.bfloat16)
    return wt, dlt, g10, ones_v, Cacc


def kernel(data, input_distros, dense_layer_weights):
    global LAST_LNY, LAST_RESULTS, _CACHED
    wt, dlt, g10, ones_v, Cacc = _host_prep(
        np.asarray(data), np.asarray(input_distros),
        np.asarray(dense_layer_weights))

    if _CACHED is None:
        _CACHED = _build_nc()
    nc = _CACHED

    in_maps = []
    for c in range(NCORES):
        in_maps.append({
            "wt": wt, "dlt": dlt, "ones": ones_v,
            "g10": np.ascontiguousarray(g10[:, :, c * BC:(c + 1) * BC]),
        })
    res = run_bass_kernel_spmd(
        nc, in_maps, core_ids=list(range(NCORES)),
        trace=bool(int(os.environ.get("KERNEL_TRACE", "0"))))
    LAST_RESULTS = res
    cs = np.concatenate([res.results[c]["colsum"].reshape(-1)
                         for c in range(NCORES)])           # (B,)
    with np.errstate(divide="ignore", invalid="ignore"):
        lnY = np.log(cs.astype(np.float64)) + Cacc
    LAST_LNY = lnY
    y = np.where(np.isfinite(lnY), np.exp(lnY), 0.0)
    y = y.astype(np.float32).reshape(B, 1)
    return y


# revision 18
# speedup vs baseline: 1.3610x; 1.3610x over previous
"""HMM window log-likelihood on 8 NeuronCores (data-parallel over batch).

Math (per batch column b): y[b] = exp(logsumexp_i x_T[b,i]) with a log-space
forward recursion; evaluated here as a linear-space BACKWARD recursion
    beta_L = 1;  c_t = em_t . beta_t;  beta_{t-1} = Wn_t^T c_t
    y[b] = sum_i c_0[i,b]
where Wn_t = row-softmax(w[t-1]) (rowsum folded on host into wt) and
em_t[i,b] = L[i, bin(b,t)] * g[t] is the emission table with per-step
rescale scalars g[t] (host-computed from batch column 0 in f64) folded in,
shipped to the device as one fp8 tensor (SBUF-resident stream, since the
DVE multiply may read at most one PSUM operand).  Device per step and per
256-column chain: c = em_sb * beta_ps (one DVE multiply, the only
elementwise op) -> beta matmul (PE).  Two independent chains pipeline the
serial recursion across PE/DVE; dummy matmuls keep the Tensor engine's
DVFS p-state high so the beta matmuls on the critical path stay short.
Device returns colsum[b]; host: lnY = log(colsum)+C.  True lnY ~ -584.6
underflows f32 to 0.0, exactly matching the reference.
"""
import sys, os
for p in ("/opt/trn_rl_repo",):
    if p not in sys.path:
        sys.path.insert(0, p)
import numpy as np
import ml_dtypes

from concourse import bass, bacc, mybir
from concourse.tile import TileContext
from concourse.bass_utils import run_bass_kernel_spmd

W, L, B, NB = 128, 256, 4096, 10
NCORES = 8
BC = B // NCORES          # 512 batch cols per core
# two independent sub-chains pipelined on the DVE (the only non-Act
# engine that can read PSUM; GpSimd has no PSUM port on TRN2)
CHAINS = (
    ("A", 0, 256),
    ("B", 256, 512),
)
TEB = 16                  # emission-block steps per DMA tile
DUMW = int(os.environ.get("KERNEL_DUMW", "160"))   # dummy matmul width
DUMN = int(os.environ.get("KERNEL_DUMN", "5"))     # dummy matmuls per step

LAST_LNY = None           # debug: device-derived lnY per batch col
LAST_RESULTS = None       # debug: raw BassKernelResults

_CACHED = None            # (nc,) build cache


def _build_nc():
    nc = bacc.Bacc("TRN2", target_bir_lowering=False, debug=False,
                   num_devices=NCORES)
    bf16, f32, fp8 = mybir.dt.bfloat16, mybir.dt.float32, mybir.dt.float8e4

    wt = nc.dram_tensor("wt", [W, L - 1, W], bf16, kind="ExternalInput")
    em = nc.dram_tensor("em", [W, L, BC], fp8, kind="ExternalInput")
    ones = nc.dram_tensor("ones", [W, 1], bf16, kind="ExternalInput")
    colsum = nc.dram_tensor("colsum", [1, BC], f32, kind="ExternalOutput")

    with TileContext(nc) as tc:
        with tc.sbuf_pool(name="sb", bufs=2) as sb, \
                tc.psum_pool(name="ps", bufs=2) as ps:
            ones_sb = sb.tile([W, 1], bf16, bufs=1)
            nc.sync.dma_start(ones_sb, ones.ap())
            dum_sb = sb.tile([W, DUMW], bf16, bufs=1)
            nc.gpsimd.memset(dum_sb, 0.0)

            # all 255 transition matrices resident; chunked DMAs in backward
            # order so the scan can start as soon as the tail chunk lands.
            # The em-block DMA for the first (highest-t) steps is issued
            # between the first wt chunks so the recursion isn't stuck
            # behind the full 8 MB wt transfer.
            wt_sb = sb.tile([W, L - 1, W], bf16, bufs=1)
            em_tiles = {}

            def dma_em_block(blk):
                em_sb = sb.tile([W, TEB, BC], fp8, tag="em", bufs=3)
                nc.sync.dma_start(
                    em_sb, em.ap()[:, blk * TEB:(blk + 1) * TEB, :])
                em_tiles[blk] = em_sb

            NBLK = L // TEB
            dma_em_block(NBLK - 1)
            for cc in range((L - 1 + 7) // 8 - 1, -1, -1):
                t0 = cc * 8
                cnt = min(8, L - 1 - t0)
                nc.sync.dma_start(wt_sb[:, t0:t0 + cnt, :],
                                  wt.ap()[:, t0:t0 + cnt, :])
                if cc == (L - 1 + 7) // 8 - 3:
                    dma_em_block(NBLK - 2)

            dum_ps = ps.tile([W, DUMW], f32, tag="dum", bufs=1)

            def dummy_mm(rhs=None):
                # p-state filler: result never read; WAW on dum_ps only.
                # Anchoring rhs to the live c tile keeps the scheduler from
                # hoisting the filler away from its step.
                nc.tensor.matmul(dum_ps, wt_sb[:, L - 2, :],
                                 dum_sb if rhs is None else rhs,
                                 start=True, stop=True,
                                 skip_group_check=True)

            # pre-ramp the PE while input DMAs land
            for _ in range(8):
                dummy_mm()

            beta_ps = {}
            cs_ps = ps.tile([1, BC], f32, tag="cs", bufs=1)
            for blk in range(NBLK - 1, -1, -1):
                if blk not in em_tiles:
                    dma_em_block(blk)
                em_sb = em_tiles.pop(blk)
                for ti in range(TEB - 1, -1, -1):
                    t = blk * TEB + ti
                    c_sb = {}
                    for name, lo, hi in CHAINS:
                        c = sb.tile([W, hi - lo], bf16, tag=f"c{name}",
                                    bufs=2)
                        if t == L - 1:
                            nc.vector.tensor_copy(c, em_sb[:, ti, lo:hi])
                        else:
                            nc.vector.tensor_mul(c, em_sb[:, ti, lo:hi],
                                                 beta_ps[name])
                        c_sb[name] = c
                    for name, lo, hi in CHAINS:
                        if t > 0:
                            b_ps = ps.tile([W, hi - lo], f32, tag=f"b{name}",
                                           bufs=2)
                            nc.tensor.matmul(b_ps, wt_sb[:, t - 1, :],
                                             c_sb[name], start=True,
                                             stop=True)
                            beta_ps[name] = b_ps
                        else:
                            nc.tensor.matmul(cs_ps[:, lo:hi], ones_sb,
                                             c_sb[name], start=True,
                                             stop=True)
                    for di in range(DUMN if DUMW else 0):
                        dummy_mm()

            cs_sb = sb.tile([1, BC], f32, bufs=1)
            nc.vector.tensor_copy(cs_sb, cs_ps)
            nc.sync.dma_start(colsum.ap(), cs_sb)
    nc.compile()
    return nc


def _host_prep(data, input_distros, dense_layer_weights):
    f64 = np.float64
    w = dense_layer_weights.astype(f64)                    # (255,W,W)
    w = w - w.max(axis=2, keepdims=True)
    we = np.exp(w)
    wn = we / we.sum(axis=2, keepdims=True)                # row-softmax
    d = input_distros.astype(f64)
    d = d - d.max(axis=1, keepdims=True)
    e = np.exp(d)
    Ls = e / e.sum(axis=1, keepdims=True)                  # (W,NB) softmax rows
    # bins exactly as reference: floor(v / 0.1) in f32
    bins = np.minimum(NB - 1, np.floor(
        data / np.float32(0.1)).astype(np.int32))          # (B,L)

    # column-0 f64 backward pass -> per-step rescale g[t], offset C
    beta = np.ones(W, dtype=f64)
    Cacc = 0.0
    g = np.ones(L, dtype=f64)
    for t in range(L - 1, 0, -1):
        c = Ls[np.arange(W), bins[0, t]] * beta
        tmp = wn[t - 1].T @ c
        f = tmp.max()
        g[t] = 1.0 / f
        Cacc += np.log(f)
        beta = tmp * g[t]

    wt = np.ascontiguousarray(
        wn.transpose(1, 0, 2)).astype(ml_dtypes.bfloat16)  # (W,255,W)

    # emission table with per-step scale folded: em[i,t,b] = Ls[i,bin]*g[t]
    emf = Ls[:, bins.T]                                    # (W, L, B) f64
    emf *= g[None, :, None]
    np.clip(emf, 0.0, 224.0, out=emf)
    em8 = emf.astype(ml_dtypes.float8_e4m3)                # (W, L, B)

    ones_v = np.ones((W, 1), dtype=ml_dtypes.bfloat16)
    return wt, em8, ones_v, Cacc


def kernel(data, input_distros, dense_layer_weights):
    global LAST_LNY, LAST_RESULTS, _CACHED
    wt, em8, ones_v, Cacc = _host_prep(
        np.asarray(data), np.asarray(input_distros),
        np.asarray(dense_layer_weights))

    if _CACHED is None:
        _CACHED = _build_nc()
    nc = _CACHED

    in_maps = []
    for c in range(NCORES):
        in_maps.append({
            "wt": wt, "ones": ones_v,
            "em": np.ascontiguousarray(em8[:, :, c * BC:(c + 1) * BC]),
        })
    res = run_bass_kernel_spmd(
        nc, in_maps, core_ids=list(range(NCORES)),
        trace=bool(int(os.environ.get("KERNEL_TRACE", "0"))))
    LAST_RESULTS = res
    cs = np.concatenate([res.results[c]["colsum"].reshape(-1)
                         for c in range(NCORES)])           # (B,)
    with np.errstate(divide="ignore", invalid="ignore"):
        lnY = np.log(cs.astype(np.float64)) + Cacc
    LAST_LNY = lnY
    y = np.where(np.isfinite(lnY), np.exp(lnY), 0.0)
    y = y.astype(np.float32).reshape(B, 1)
    return y
